# revision 1
# baseline (speedup 1.0000x reference)
"""Bahdanau attention Trainium2 kernel.

Contract: kernel(**inputs) takes FULL unsharded inputs (numpy arrays, keys as
in setup_inputs) and returns the FULL (B, T, H) float32 context output.

Sharding: over T (query timesteps). Each of the 8 cores processes all B=8
batches but only T/8 = 16 timesteps. This keeps the SPMD program identical
across cores while letting per-batch src_lengths clamp the score/softmax
work at compile time (identical clamps on every core).

Math per (b, t): scores[s] = v . tanh(Ws q_t + Wh h_s + (Ws_b + Wh_b)),
softmax over s < len_b (v_b dropped: softmax shift-invariant), context =
attn @ enc. Layouts keep the hidden dim on SBUF partitions (4 chunks of 128)
so the q_t + h_s broadcast-add is one stride-0 tensor_tensor per (b, chunk)
on DVE; ADD_FUSE_K of the 16 t-slices instead fuse the add into ACT's tanh
via the per-partition bias operand (balances DVE vs ACT, both near-saturated).
The v-reduction over the hidden dim runs on the PE with a host-built block of
per-t selection weights (column t = v chunk, rest 0) accumulating into one
(16, len) PSUM tile; softmax uses exact lengths (no masking; v_b cancels) and
skips the max-subtraction (scores are bounded by ||v||_1, exp is fp32-safe)
with exp+row-sum fused via ACT accum_out, and the 1/sum normalization is folded
into the context's PSUM->SBUF copy. All matmul operands are bf16 (fp32
matmuls get split into two HW passes); PSUM accumulation and softmax
statistics stay fp32. Batches are processed longest-first so the pipeline
tail is short, and inputs arrive as a handful of large packed DMAs.
"""

import sys

if "/opt/trn_rl_repo" not in sys.path:
    sys.path.insert(0, "/opt/trn_rl_repo")

import numpy as np

B, T, S, H = 8, 128, 256, 512
NCORES = 8
TSH = T // NCORES  # 16 timesteps per core
KC = H // 128  # 4 contraction chunks

# Per (b, chunk): the first ADD_FUSE_K of the 16 t-slices compute
# tanh(h + q_t) fully on ACT (fused bias), the rest get a DVE
# tensor_scalar add followed by one batched ACT tanh. Balances DVE vs ACT.
ADD_FUSE_K = 2
# Engine routing for the non-fused add slices of unit u=(bi*KC+c):
# u % GP_EVERY == GP_EVERY-1 -> one gpsimd tensor_tensor (f32, own lanes);
# otherwise DVE; DVE_TS chooses per-t tensor_scalar (single read port,
# avoids the DVE/GpSimd shared-port lock) vs one batched tensor_tensor.
GP_EVERY = 0
DVE_TS = False

_CACHE: dict = {}


def _build(lengths):
    import concourse.bass as bass
    import concourse.tile as tile
    import concourse.mybir as mybir
    from concourse import bacc
    from concourse.masks import make_identity

    f32 = mybir.dt.float32
    bf16 = mybir.dt.bfloat16
    nc = bacc.Bacc("TRN2", target_bir_lowering=False, debug=False)

    qT_d = nc.dram_tensor("qT", [128, KC, NCORES * TSH], bf16, kind="ExternalInput")
    encT_d = nc.dram_tensor("encT", [128, KC, B, S], bf16, kind="ExternalInput")
    enc_d = nc.dram_tensor("enc", [128, S // 128, B, H], bf16, kind="ExternalInput")
    wwT_d = nc.dram_tensor("wwT", [128, 2 * KC, H], bf16, kind="ExternalInput")
    bias_d = nc.dram_tensor("bias", [128, KC], f32, kind="ExternalInput")
    vsel_d = nc.dram_tensor("vsel", [128, KC, TSH, TSH], bf16, kind="ExternalInput")
    out_d = nc.dram_tensor("out", [B, TSH, H], f32, kind="ExternalOutput")

    AT = mybir.AluOpType
    AF = mybir.ActivationFunctionType
    AX = mybir.AxisListType

    with tile.TileContext(nc) as tc:
        with (
            tc.tile_pool(name="const", bufs=1) as const,
            tc.tile_pool(name="enctp", bufs=3) as enctp,
            tc.tile_pool(name="htp", bufs=2) as htp,
            tc.tile_pool(name="addp", bufs=3) as addp,
            tc.tile_pool(name="addfp", bufs=2) as addfp,
            tc.tile_pool(name="tanp", bufs=3) as tanp,
            tc.tile_pool(name="attnp", bufs=2) as attnp,
            tc.tile_pool(name="smallp", bufs=2) as smallp,
            tc.tile_pool(name="attntp", bufs=2) as attntp,
            tc.tile_pool(name="encbp", bufs=3) as encbp,
            tc.tile_pool(name="outp", bufs=2) as outp,
            tc.tile_pool(name="pjh", bufs=3, space="PSUM") as pjh,
            tc.tile_pool(name="scps", bufs=3, space="PSUM") as scps,
            tc.tile_pool(name="miscp", bufs=1, space="PSUM") as miscp,
            tc.tile_pool(name="ctxp", bufs=1, space="PSUM") as ctxp,
        ):
            border = sorted(range(B), key=lambda i: -int(lengths[i]))

            # ---- constants / weights; critical-path DMAs first ----
            wwT = const.tile([128, 2 * KC, H], bf16)
            whT = wwT[:, :KC, :]
            wsT = wwT[:, KC:, :]
            nc.sync.dma_start(whT, wwT_d.ap()[:, :KC, :])
            b0 = border[0]
            L0 = int(lengths[b0])
            encT_first = enctp.tile([128, KC, S], bf16)
            nc.sync.dma_start(
                encT_first[:, :, :L0], encT_d.ap()[:, :, b0, :L0]
            )
            nc.sync.dma_start(wsT, wwT_d.ap()[:, KC:, :])
            qin = const.tile([128, KC, NCORES * TSH], bf16)
            nc.sync.dma_start(qin[:], qT_d.ap())
            vsel = const.tile([128, KC, TSH, TSH], bf16)
            nc.sync.dma_start(vsel[:], vsel_d.ap())
            bias = const.tile([128, KC], f32)
            nc.sync.dma_start(bias[:], bias_d.ap())
            ident = const.tile([TSH, TSH], bf16)
            make_identity(nc, ident[:])

            # ---- phase A: q projection (combined bias folded in) ----
            qT_sb = const.tile([128, KC, NCORES * TSH], f32)
            for oc in range(KC):
                qps = miscp.tile([128, NCORES * TSH], f32, tag="mshare")
                for kc in range(KC):
                    nc.tensor.matmul(
                        qps[:],
                        wsT[:, kc, oc * 128:(oc + 1) * 128],
                        qin[:, kc, :],
                        start=(kc == 0),
                        stop=(kc == KC - 1),
                    )
                nc.vector.tensor_scalar_add(
                    qT_sb[:, oc, :], qps[:], bias[:, oc:oc + 1]
                )

            # ---- phase B: per batch, longest first (short tail) ----
            for bi, b in enumerate(border):
                L = int(lengths[b])
                nsc = (L + 127) // 128

                if bi == 0:
                    encT_b = encT_first
                else:
                    encT_b = enctp.tile([128, KC, S], bf16)
                    nc.sync.dma_start(
                        encT_b[:, :, :L], encT_d.ap()[:, :, b, :L]
                    )

                # h projection (bias lives in qT_sb) -> hT_b in SBUF bf16
                hT_b = htp.tile([128, KC, S], bf16)
                for oc in range(KC):
                    hps = pjh.tile([128, S], f32)
                    for kc in range(KC):
                        nc.tensor.matmul(
                            hps[:, :L],
                            whT[:, kc, oc * 128:(oc + 1) * 128],
                            encT_b[:, kc, :L],
                            start=(kc == 0),
                            stop=(kc == KC - 1),
                        )
                    nc.vector.tensor_copy(hT_b[:, oc, :L], hps[:, :L])

                # scores: tanh(q_t + h_s) reduced against v
                sc_ps = scps.tile([TSH, S], f32)
                kb = ADD_FUSE_K + (1 if L >= 190 else 0) - (1 if L < 75 else 0)
                for c in range(KC):
                    k = kb
                    tanhout = tanp.tile([128, TSH, S], bf16)
                    for t in range(k):
                        nc.scalar.activation(
                            tanhout[:, t, :L],
                            hT_b[:, c, :L],
                            AF.Tanh,
                            bias=qT_sb[:, c, b * TSH + t:b * TSH + t + 1],
                        )
                    u = bi * KC + c
                    use_gp = GP_EVERY > 0 and u % GP_EVERY == GP_EVERY - 1
                    if k < TSH:
                        ntv = TSH - k
                        if use_gp:
                            addf = addfp.tile([128, TSH, S], f32)
                            q_bc = qT_sb[:, c, b * TSH + k:(b + 1) * TSH][
                                :, :, None
                            ].to_broadcast((128, ntv, L))
                            h_bc = hT_b[:, c, :L][:, None, :].to_broadcast(
                                (128, ntv, L)
                            )
                            nc.gpsimd.tensor_tensor(
                                addf[:, k:, :L], q_bc, h_bc, AT.add
                            )
                            nc.scalar.activation(
                                tanhout[:, k:, :L], addf[:, k:, :L], AF.Tanh
                            )
                        else:
                            addout = addp.tile([128, TSH, S], bf16)
                            if DVE_TS:
                                for t in range(k, TSH):
                                    nc.vector.tensor_scalar_add(
                                        addout[:, t, :L],
                                        hT_b[:, c, :L],
                                        qT_sb[:, c, b * TSH + t:b * TSH + t + 1],
                                    )
                            else:
                                q_bc = qT_sb[:, c, b * TSH + k:(b + 1) * TSH][
                                    :, :, None
                                ].to_broadcast((128, ntv, L))
                                h_bc = hT_b[:, c, :L][:, None, :].to_broadcast(
                                    (128, ntv, L)
                                )
                                nc.vector.tensor_tensor(
                                    addout[:, k:, :L], q_bc, h_bc, AT.add
                                )
                            nc.scalar.activation(
                                tanhout[:, k:, :L], addout[:, k:, :L], AF.Tanh
                            )
                    for t in range(TSH):
                        nc.tensor.matmul(
                            sc_ps[:, :L],
                            vsel[:, c, t, :],
                            tanhout[:, t, :L],
                            start=(c == 0 and t == 0),
                            stop=(c == KC - 1 and t == TSH - 1),
                        )

                # softmax over s < L (exact length; no masking needed).
                # No max-subtraction: |score| <= ||v||_1 ~ 11, exp() is safe
                # in fp32, and softmax ratios are identical -- this removes a
                # DVE reduce and shortens the per-batch serial chain.
                attn = attnp.tile([TSH, S], bf16)
                nc.scalar.activation(
                    attn[:, :L],
                    sc_ps[:, :L],
                    AF.Exp,
                )
                sumexp = smallp.tile([TSH, 1], f32)
                nc.vector.tensor_reduce(
                    sumexp[:], attn[:, :L], axis=AX.X, op=AT.add
                )
                rsum = smallp.tile([TSH, 1], f32)
                nc.vector.reciprocal(rsum[:], sumexp[:])

                # attn^T (s on partitions), zero-padded to S
                attnT = attntp.tile([128, S // 128, TSH], bf16)
                nc.gpsimd.memset(attnT[:], 0.0)
                for sc in range(nsc):
                    cl = min(128, L - sc * 128)
                    tps = miscp.tile([128, TSH], bf16, tag="mshare")
                    nc.tensor.transpose(
                        tps[:cl, :], attn[:, sc * 128:sc * 128 + cl], ident[:]
                    )
                    nc.vector.tensor_copy(attnT[:cl, sc, :], tps[:cl, :])

                # context = attn @ enc  (padded rows of attnT are zero)
                enc_b = encbp.tile([128, S // 128, H], bf16)
                nc.sync.dma_start(enc_b[:], enc_d.ap()[:, :, b, :])
                ctx_ps = ctxp.tile([TSH, H], f32)
                for sc in range(S // 128):
                    nc.tensor.matmul(
                        ctx_ps[:],
                        attnT[:, sc, :],
                        enc_b[:, sc, :],
                        start=(sc == 0),
                        stop=(sc == S // 128 - 1),
                    )
                ctx_sb = outp.tile([TSH, H], f32)
                nc.vector.tensor_scalar_mul(ctx_sb[:], ctx_ps[:], rsum[:])
                nc.sync.dma_start(out_d.ap()[b], ctx_sb[:])

    nc.compile()
    return nc


def _prep_inputs(query, encoder_outputs, Ws_w, Ws_b, Wh_w, Wh_b, v_w):
    """Host-side layout staging (no math beyond the bias sum)."""
    import ml_dtypes

    bf = ml_dtypes.bfloat16
    query = np.asarray(query, dtype=np.float32)
    enc32 = np.asarray(encoder_outputs, dtype=np.float32)
    enc = np.ascontiguousarray(enc32.astype(bf))
    wsT = np.ascontiguousarray(np.asarray(Ws_w, dtype=np.float32).T.astype(bf))
    whT = np.ascontiguousarray(np.asarray(Wh_w, dtype=np.float32).T.astype(bf))
    bias = np.ascontiguousarray(
        (np.asarray(Ws_b, dtype=np.float32) + np.asarray(Wh_b, dtype=np.float32))
        .reshape(KC, 128)
        .T
    )
    v = np.asarray(v_w, dtype=np.float32)[0]
    vsel = np.zeros((128, KC, TSH, TSH), dtype=np.float32)
    for c in range(KC):
        for t in range(TSH):
            vsel[:, c, t, t] = v[c * 128:(c + 1) * 128]
    vsel = vsel.astype(bf)
    # encT[p, c, b, s] = enc[b, s, c*128+p]
    encT = np.ascontiguousarray(
        enc32.reshape(B, S, KC, 128).transpose(3, 2, 0, 1).astype(bf)
    )
    # enc_nat[p, sc, b, h] = enc[b, sc*128+p, h]
    enc_nat = np.ascontiguousarray(
        enc32.reshape(B, S // 128, 128, H).transpose(2, 1, 0, 3).astype(bf)
    )
    # wwT[p, j, o]: j<KC -> Wh_w.T chunks, j>=KC -> Ws_w.T chunks
    wwT = np.ascontiguousarray(
        np.concatenate(
            [whT.reshape(KC, 128, H), wsT.reshape(KC, 128, H)], axis=0
        ).transpose(1, 0, 2)
    )

    in_maps = []
    for core in range(NCORES):
        qsh = query[:, core * TSH:(core + 1) * TSH, :]  # (B, TSH, H)
        # qT[p, c, bt] = qsh[b, t, c*128+p]
        qT = np.ascontiguousarray(
            qsh.reshape(B * TSH, KC, 128).transpose(2, 1, 0).astype(bf)
        )
        in_maps.append(
            {
                "qT": qT,
                "encT": encT,
                "enc": enc_nat,
                "wwT": wwT,
                "bias": bias,
                "vsel": vsel,
            }
        )
    return in_maps


def kernel(query, encoder_outputs, src_lengths, Ws_w, Ws_b, Wh_w, Wh_b, v_w, v_b):
    from concourse import bass_utils

    lengths = tuple(int(x) for x in np.asarray(src_lengths).reshape(-1))
    assert len(lengths) == B
    if lengths not in _CACHE:
        _CACHE[lengths] = _build(lengths)
    nc = _CACHE[lengths]

    in_maps = _prep_inputs(query, encoder_outputs, Ws_w, Ws_b, Wh_w, Wh_b, v_w)
    res = bass_utils.run_bass_kernel_spmd(nc, in_maps, core_ids=list(range(NCORES)))

    out = np.empty((B, T, H), dtype=np.float32)
    for core in range(NCORES):
        out[:, core * TSH:(core + 1) * TSH, :] = res.results[core]["out"]
    return out

